# revision 1
# baseline (speedup 1.0000x reference)
"""GNN edge-scorer (MLPPredictor) Trainium2 kernel.

score[e, :] = h[src[e]] @ Wu.T + h[dst[e]] @ Wv.T + b
  h   [100000, 128] f32
  src/dst [600000] i64
  W   [64, 256] f32  (Wu = W[:, :128], Wv = W[:, 128:])
  out [600000, 64] f32

Strategy (8 cores, edge-sharded):
  - Host: split W into WuT/WvT [128, 64], cast indices to i32, pad edges,
    lay indices out as [128, n_blocks*S] so edge (t*2048 + p*16 + s) sits at
    partition p, column t*16+s.
  - Device (per core, per 2048-edge block of S=16 subtiles):
      * per 128-edge subtile: indirect-DMA gather of h[src] and h[dst] rows
        (HW contract: one offset per partition per instruction, 512 B rows),
        PE transpose [128e, 128d] -> [128d, 128e], ACT copies PSUM->SBUF,
        two fp32 matmuls accumulate psum[128e, 64] = g_uT.T@WuT + g_vT.T@WvT,
        DVE adds broadcast bias into the score slab column
      * one contiguous 512 KB store per block
"""

import numpy as np

import concourse.bacc as bacc
import concourse.bass as bass
import concourse.mybir as mybir
import concourse.tile as tile
from concourse.bass_utils import run_bass_kernel_spmd
from concourse.masks import make_identity

N_CORES = 8
N_NODES = 100000
N_EDGES = 600000
D = 128          # feature dim
C = 64           # output classes
S_SUB = 16       # 128-edge subtiles per store block
BLOCK_EDGES = 128 * S_SUB                      # 2048
EDGES_PER_CORE = -(-N_EDGES // N_CORES)        # 75000
N_BLOCKS = -(-EDGES_PER_CORE // BLOCK_EDGES)   # 37
EDGES_PER_CORE_PAD = N_BLOCKS * BLOCK_EDGES    # 75776

_F32 = mybir.dt.float32
_I32 = mybir.dt.int32

_CACHE: dict = {}


def build_nc():
    if "nc" in _CACHE:
        return _CACHE["nc"]
    nc = bacc.Bacc("TRN2", target_bir_lowering=False)
    h = nc.dram_tensor("h", [N_NODES, D], _F32, kind="ExternalInput")
    isrc = nc.dram_tensor("isrc", [128, N_BLOCKS * S_SUB], _I32, kind="ExternalInput")
    idst = nc.dram_tensor("idst", [128, N_BLOCKS * S_SUB], _I32, kind="ExternalInput")
    wut = nc.dram_tensor("wut", [D, C], _F32, kind="ExternalInput")
    wvt = nc.dram_tensor("wvt", [D, C], _F32, kind="ExternalInput")
    bvec = nc.dram_tensor("bvec", [128, C], _F32, kind="ExternalInput")
    out = nc.dram_tensor("out", [N_BLOCKS, 128, S_SUB * C], _F32, kind="ExternalOutput")

    with tile.TileContext(nc) as tc:
        with (
            tc.tile_pool(name="const", bufs=1) as cpool,
            tc.tile_pool(name="gather", bufs=12) as gpool,
            tc.tile_pool(name="tr", bufs=4) as trpool,
            tc.tile_pool(name="score", bufs=2) as spool,
            tc.tile_pool(name="psum_t", bufs=4, space="PSUM") as ptpool,
            tc.tile_pool(name="psum_s", bufs=2, space="PSUM") as pspool,
        ):
            ident = cpool.tile([128, 128], _F32)
            make_identity(nc, ident[:])
            wut_sb = cpool.tile([D, C], _F32)
            nc.sync.dma_start(out=wut_sb[:], in_=wut[:, :])
            wvt_sb = cpool.tile([D, C], _F32)
            nc.sync.dma_start(out=wvt_sb[:], in_=wvt[:, :])
            bias_b = cpool.tile([128, C], _F32)
            nc.sync.dma_start(out=bias_b[:], in_=bvec[:, :])
            isrc_sb = cpool.tile([128, N_BLOCKS * S_SUB], _I32)
            nc.sync.dma_start(out=isrc_sb[:], in_=isrc[:, :])
            idst_sb = cpool.tile([128, N_BLOCKS * S_SUB], _I32)
            nc.sync.dma_start(out=idst_sb[:], in_=idst[:, :])

            for t in range(N_BLOCKS):
                score = spool.tile([128, S_SUB * C], _F32, tag="sc")
                for s in range(S_SUB):
                    col = t * S_SUB + s
                    g_u = gpool.tile([128, D], _F32, tag="gu")
                    nc.gpsimd.indirect_dma_start(
                        out=g_u[:],
                        out_offset=None,
                        in_=h[:, :],
                        in_offset=bass.IndirectOffsetOnAxis(
                            ap=isrc_sb[:, col:col + 1], axis=0
                        ),
                    )
                    g_v = gpool.tile([128, D], _F32, tag="gv")
                    nc.gpsimd.indirect_dma_start(
                        out=g_v[:],
                        out_offset=None,
                        in_=h[:, :],
                        in_offset=bass.IndirectOffsetOnAxis(
                            ap=idst_sb[:, col:col + 1], axis=0
                        ),
                    )
                    pt_u = ptpool.tile([128, 128], _F32, tag="pt")
                    nc.tensor.transpose(pt_u[:], g_u[:], ident[:])
                    gut = trpool.tile([128, 128], _F32, tag="gt")
                    nc.scalar.copy(out=gut[:], in_=pt_u[:])
                    pt_v = ptpool.tile([128, 128], _F32, tag="pt")
                    nc.tensor.transpose(pt_v[:], g_v[:], ident[:])
                    gvt = trpool.tile([128, 128], _F32, tag="gt")
                    nc.scalar.copy(out=gvt[:], in_=pt_v[:])
                    ps = pspool.tile([128, C], _F32, tag="ps")
                    nc.tensor.matmul(ps[:], lhsT=gut[:], rhs=wut_sb[:], start=True, stop=False)
                    nc.tensor.matmul(ps[:], lhsT=gvt[:], rhs=wvt_sb[:], start=False, stop=True)
                    nc.vector.tensor_tensor(
                        out=score[:, s * C:(s + 1) * C],
                        in0=ps[:],
                        in1=bias_b[:],
                        op=mybir.AluOpType.add,
                    )
                nc.sync.dma_start(out=out[t, :, :], in_=score[:])

    nc.finalize()
    _CACHE["nc"] = nc
    return nc


def _prep_index_chunk(idx_pad: np.ndarray, core: int) -> np.ndarray:
    chunk = idx_pad[core * EDGES_PER_CORE_PAD:(core + 1) * EDGES_PER_CORE_PAD]
    # edge (t*2048 + p*16 + s) -> [p, t*16 + s]
    return np.ascontiguousarray(
        chunk.reshape(N_BLOCKS, 128, S_SUB).transpose(1, 0, 2).reshape(128, N_BLOCKS * S_SUB)
    )


def make_in_maps(h, src, dst, W, b):
    h = np.ascontiguousarray(np.asarray(h, dtype=np.float32))
    W = np.asarray(W, dtype=np.float32)
    b = np.asarray(b, dtype=np.float32)
    wut = np.ascontiguousarray(W[:, :D].T)   # [128, 64]
    wvt = np.ascontiguousarray(W[:, D:].T)   # [128, 64]
    bvec = np.ascontiguousarray(np.broadcast_to(b.reshape(1, C), (128, C)))

    e_pad = N_CORES * EDGES_PER_CORE_PAD
    src_pad = np.zeros(e_pad, dtype=np.int32)
    dst_pad = np.zeros(e_pad, dtype=np.int32)
    src_pad[:N_EDGES] = np.asarray(src).astype(np.int32)
    dst_pad[:N_EDGES] = np.asarray(dst).astype(np.int32)

    in_maps = []
    for core in range(N_CORES):
        in_maps.append(
            {
                "h": h,
                "isrc": _prep_index_chunk(src_pad, core),
                "idst": _prep_index_chunk(dst_pad, core),
                "wut": wut,
                "wvt": wvt,
                "bvec": bvec,
            }
        )
    return in_maps


def assemble_output(results) -> np.ndarray:
    per_core = [
        np.asarray(r["out"], dtype=np.float32).reshape(EDGES_PER_CORE_PAD, C)
        for r in results
    ]
    return np.concatenate(per_core, axis=0)[:N_EDGES]


def run(h, src, dst, W, b, **spmd_kwargs):
    nc = build_nc()
    in_maps = make_in_maps(h, src, dst, W, b)
    res = run_bass_kernel_spmd(nc, in_maps, core_ids=list(range(N_CORES)), **spmd_kwargs)
    return assemble_output(res.results), res


def kernel(h, src, dst, W, b):
    out, _ = run(h, src, dst, W, b)
    return out



# revision 10
# speedup vs baseline: 1.4369x; 1.4369x over previous
"""GNN edge-scorer (MLPPredictor) Trainium2 kernel.

score[e, :] = h[src[e]] @ Wu.T + h[dst[e]] @ Wv.T + b
  h   [100000, 128] f32
  src/dst [600000] i64
  W   [64, 256] f32  (Wu = W[:, :128], Wv = W[:, 128:])
  out [600000, 64] f32

Strategy (8 cores, edge-sharded, fp16 internally):
  - Host: cast h -> fp16 [100096, 128]; per core sort its 75k edges by
    (src>>15, dst>>15) into 16 bins so node indices fit int16 relative to a
    32768-row table base; pad bins to 128 (+last bin to make total %512);
    lay per-chunk indices as [16, L/16] int16 in an SBUF canvas.
  - Device (per core): for each 8192-edge slab:
      * transpose-mode dma_gather pulls h16 rows (256B) into
        guT/gvT [128 feat, slab] fp16 directly (xbar transpose in DMA;
        one instruction per multi-thousand-edge chunk -> SWDGE cheap)
      * per 512-edge group: psum[64, 512] = WuT.T@guT + WvT.T@gvT (+ b)
        with stationary [128, 64] weights, ACT copies psum -> fp16 score
      * one [64, slab] fp16 store (16 KB/partition descriptors)
  - Host: transpose [64, S_PAD] -> [S_PAD, 64], un-permute, cast f32.
"""

import numpy as np

import concourse.bacc as bacc
import concourse.bass as bass
import concourse.mybir as mybir
import concourse.tile as tile
from concourse.bass_utils import run_bass_kernel_spmd

N_CORES = 8
N_NODES = 100000
N_EDGES = 600000
D = 128
C = 64
N_NODES_PAD = 100096          # mult of 128
EDGES_PER_CORE = N_EDGES // N_CORES
BIN_SHIFT = 15                # 32768-node table windows (int16 idx)
N_SRC_BINS = (N_NODES >> BIN_SHIFT) + 1   # 4
N_BINS = N_SRC_BINS * N_SRC_BINS          # 16
SLAB = 7680                   # edges per score slab / gather target tile
MM = 512                      # edges per matmul group (one PSUM bank)
# One gather needs num_idxs/16 + 2 descriptor-ring slots per engine and the
# ring holds ~1024; 7680 -> 482 slots. Requires single_packet=False (a
# coalesced packet is limited to ~4KB/16 descs per engine).
MAX_GATHER = 7680
# All transpose-gathers stay on ONE SWDGE queue: per-engine queue rings drain
# FIFO, so two gathers' rx descriptors never interleave in the engine's xbar
# transpose stream (interleave across queues corrupts the 256B tile state).
N_QUEUES = 1

_F32 = mybir.dt.float32
_F16 = mybir.dt.float16
_I16 = mybir.dt.int16

_CACHE: dict = {}


def _plan_chunks(caps):
    """Static per-core slot plan from bin caps (same caps on all cores).

    Returns (s_pad, slabs, u_chunks, v_chunks, total_idx_cols) where each
    chunk is (slab_idx, dst_off_in_slab, length, table_base_row, idx_col0).
    Chunks never cross slab boundaries and are 128-aligned.
    """
    s_pad = int(sum(caps))
    assert s_pad % MM == 0
    bin_start = np.concatenate([[0], np.cumsum(caps)]).astype(int)

    slabs = []
    off = 0
    while off < s_pad:
        slabs.append(min(SLAB, s_pad - off))
        off += SLAB

    col = [0]

    def split(run_start, run_len, base_row):
        out = []
        pos = run_start
        end = run_start + run_len
        while pos < end:
            slab_i = pos // SLAB
            slab_end = (slab_i + 1) * SLAB
            ln = min(end, slab_end) - pos
            # cap at MAX_GATHER (SWDGE ring capacity), keep 128-aligned
            n_pieces = -(-ln // MAX_GATHER)
            piece = -(-(ln // 128) // n_pieces) * 128
            while ln > 0:
                p_ln = min(piece, ln)
                assert p_ln % 128 == 0 and pos % 128 == 0 and p_ln <= MAX_GATHER
                out.append((slab_i, pos - slab_i * SLAB, p_ln, base_row, col[0]))
                col[0] += p_ln // 16
                pos += p_ln
                ln -= p_ln
        return out

    u_chunks = []
    for bu in range(N_SRC_BINS):
        run_start = bin_start[bu * N_SRC_BINS]
        run_len = bin_start[(bu + 1) * N_SRC_BINS] - run_start
        if run_len:
            u_chunks += split(run_start, run_len, bu << BIN_SHIFT)
    v_chunks = []
    for k in range(N_BINS):
        if caps[k]:
            v_chunks += split(bin_start[k], caps[k], (k % N_SRC_BINS) << BIN_SHIFT)
    return s_pad, slabs, u_chunks, v_chunks, col[0]


def build_nc(caps, use_bias):
    key = (tuple(caps), use_bias)
    if key in _CACHE:
        return _CACHE[key]
    s_pad, slabs, u_chunks, v_chunks, idx_cols = _plan_chunks(caps)

    nc = bacc.Bacc("TRN2", target_bir_lowering=False, num_swdge_queues=N_QUEUES)
    h16 = nc.dram_tensor("h16", [N_NODES_PAD, D], _F16, kind="ExternalInput")
    idx = nc.dram_tensor("idx", [128, idx_cols], _I16, kind="ExternalInput")
    wut = nc.dram_tensor("wut", [D, C], _F16, kind="ExternalInput")
    wvt = nc.dram_tensor("wvt", [D, C], _F16, kind="ExternalInput")
    brow = nc.dram_tensor("brow", [1, C], _F16, kind="ExternalInput")
    out = nc.dram_tensor("out", [C, s_pad], _F16, kind="ExternalOutput")

    by_slab_u = {}
    for ch in u_chunks:
        by_slab_u.setdefault(ch[0], []).append(ch)
    by_slab_v = {}
    for ch in v_chunks:
        by_slab_v.setdefault(ch[0], []).append(ch)

    with tile.TileContext(nc) as tc:
        with (
            tc.tile_pool(name="const", bufs=1) as cpool,
            tc.tile_pool(name="gather", bufs=2) as gpool,
            tc.tile_pool(name="score", bufs=2) as spool,
            tc.tile_pool(name="psum", bufs=8, space="PSUM") as ppool,
        ):
            wut_sb = cpool.tile([D, C], _F16)
            nc.sync.dma_start(out=wut_sb[:], in_=wut[:, :])
            wvt_sb = cpool.tile([D, C], _F16)
            nc.sync.dma_start(out=wvt_sb[:], in_=wvt[:, :])
            idx_sb = cpool.tile([128, idx_cols], _I16)
            nc.sync.dma_start(out=idx_sb[:], in_=idx[:, :])
            if use_bias:
                bias_sb = cpool.tile([1, C], _F16)
                nc.sync.dma_start(out=bias_sb[:], in_=brow[:, :])
                ones_sb = cpool.tile([1, MM], _F16)
                nc.vector.memset(ones_sb[:], 1.0)

            qn = 0
            for s, slab_len in enumerate(slabs):
                guT = gpool.tile([128, 1, SLAB], _F16, tag="gu")
                gvT = gpool.tile([128, 1, SLAB], _F16, tag="gv")
                for tgt, chunks in ((guT, by_slab_u[s]), (gvT, by_slab_v[s])):
                    for (_si, doff, ln, base, c0) in chunks:
                        hi = min(base + (1 << BIN_SHIFT), N_NODES_PAD)
                        nc.gpsimd.dma_gather(
                            tgt[:, :, doff:doff + ln],
                            h16[base:hi, :],
                            idx_sb[:, c0:c0 + ln // 16],
                            ln,
                            ln,
                            D,
                            transpose=True,
                            single_packet=False,
                            queue_num=qn,
                        )
                        qn = (qn + 1) % N_QUEUES
                score = spool.tile([C, SLAB], _F16, tag="sc")
                for j in range(slab_len // MM):
                    ps = ppool.tile([C, MM], _F32, tag="ps")
                    nc.tensor.matmul(
                        ps[:], lhsT=wut_sb[:], rhs=guT[:, 0, j * MM:(j + 1) * MM],
                        start=True, stop=False,
                    )
                    nc.tensor.matmul(
                        ps[:], lhsT=wvt_sb[:], rhs=gvT[:, 0, j * MM:(j + 1) * MM],
                        start=False, stop=not use_bias,
                    )
                    if use_bias:
                        nc.tensor.matmul(
                            ps[:], lhsT=bias_sb[:], rhs=ones_sb[:],
                            start=False, stop=True,
                        )
                    nc.scalar.copy(out=score[:, j * MM:(j + 1) * MM], in_=ps[:])
                nc.sync.dma_start(
                    out=out[:, s * SLAB:s * SLAB + slab_len],
                    in_=score[:, :slab_len],
                )

    nc.finalize()
    _CACHE[key] = (nc, s_pad, u_chunks, v_chunks, idx_cols)
    return _CACHE[key]


def _bin_core(src_c, dst_c):
    bu = src_c >> BIN_SHIFT
    bv = dst_c >> BIN_SHIFT
    key = bu * N_SRC_BINS + bv
    order = np.argsort(key, kind="stable")
    counts = np.bincount(key, minlength=N_BINS)
    return order, counts


def make_in_maps(h, src, dst, W, b):
    h = np.asarray(h, dtype=np.float32)
    W = np.asarray(W, dtype=np.float32)
    b = np.asarray(b, dtype=np.float32)
    src = np.asarray(src).astype(np.int64)
    dst = np.asarray(dst).astype(np.int64)

    h16 = np.zeros((N_NODES_PAD, D), dtype=np.float16)
    h16[:N_NODES] = h.astype(np.float16)
    wut = np.ascontiguousarray(W[:, :D].T).astype(np.float16)
    wvt = np.ascontiguousarray(W[:, D:].T).astype(np.float16)
    brow = b.reshape(1, C).astype(np.float16)
    use_bias = bool(np.any(b))

    per_core = []
    max_counts = np.zeros(N_BINS, dtype=np.int64)
    for c in range(N_CORES):
        sl = slice(c * EDGES_PER_CORE, (c + 1) * EDGES_PER_CORE)
        order, counts = _bin_core(src[sl], dst[sl])
        per_core.append((order, counts))
        max_counts = np.maximum(max_counts, counts)

    caps = ((max_counts + 127) // 128) * 128
    total = int(caps.sum())
    pad_tail = (-total) % MM
    caps[-1] += pad_tail  # bin 15 always exists structurally
    caps = caps.astype(int)

    (nc, s_pad, u_chunks, v_chunks, idx_cols) = build_nc(tuple(caps), use_bias)
    bin_start = np.concatenate([[0], np.cumsum(caps)]).astype(int)

    in_maps = []
    orig_ids = []
    for c in range(N_CORES):
        e0 = c * EDGES_PER_CORE
        order, counts = per_core[c]
        src_c = src[e0:e0 + EDGES_PER_CORE][order]
        dst_c = dst[e0:e0 + EDGES_PER_CORE][order]
        key_sorted = (src_c >> BIN_SHIFT) * N_SRC_BINS + (dst_c >> BIN_SHIFT)

        u_slot = np.zeros(s_pad, dtype=np.int16)
        v_slot = np.zeros(s_pad, dtype=np.int16)
        oid = np.full(s_pad, -1, dtype=np.int64)
        pos = 0
        for k in range(N_BINS):
            n_k = int(counts[k])
            b0 = bin_start[k]
            u_slot[b0:b0 + n_k] = (src_c[pos:pos + n_k] & ((1 << BIN_SHIFT) - 1)).astype(np.int16)
            v_slot[b0:b0 + n_k] = (dst_c[pos:pos + n_k] & ((1 << BIN_SHIFT) - 1)).astype(np.int16)
            assert (key_sorted[pos:pos + n_k] == k).all()
            oid[b0:b0 + n_k] = e0 + order[pos:pos + n_k]
            pos += n_k

        canvas = np.zeros((16, idx_cols), dtype=np.int16)
        for chunks, slot in ((u_chunks, u_slot), (v_chunks, v_slot)):
            for (si, doff, ln, _base, c0) in chunks:
                g0 = si * SLAB + doff
                canvas[:, c0:c0 + ln // 16] = slot[g0:g0 + ln].reshape(ln // 16, 16).T
        # idx rows are read per-Q7-core from its own 16-partition group:
        # replicate the block across all 8 groups.
        canvas = np.tile(canvas, (8, 1))

        orig_ids.append(oid)
        in_maps.append(
            {"h16": h16, "idx": canvas, "wut": wut, "wvt": wvt, "brow": brow}
        )
    return nc, in_maps, orig_ids, s_pad


def assemble_output(results, orig_ids):
    final = np.empty((N_EDGES, C), dtype=np.float32)
    for res, oid in zip(results, orig_ids):
        scores = np.asarray(res["out"]).T.astype(np.float32)  # [s_pad, C]
        valid = oid >= 0
        final[oid[valid]] = scores[valid]
    return final


def run(h, src, dst, W, b, **spmd_kwargs):
    nc, in_maps, orig_ids, _ = make_in_maps(h, src, dst, W, b)
    res = run_bass_kernel_spmd(nc, in_maps, core_ids=list(range(N_CORES)), **spmd_kwargs)
    return assemble_output(res.results, orig_ids), res


def kernel(h, src, dst, W, b):
    out, _ = run(h, src, dst, W, b)
    return out


# revision 13
# speedup vs baseline: 2.5190x; 1.7530x over previous
"""GNN edge-scorer (MLPPredictor) Trainium2 kernel.

score[e, :] = h[src[e]] @ Wu.T + h[dst[e]] @ Wv.T + b
  h   [100000, 128] f32
  src/dst [600000] i64
  W   [64, 256] f32  (Wu = W[:, :128], Wv = W[:, 128:])
  out [600000, 64] f32

Strategy (8 cores, edge-sharded, fp16 internally):
  - Host: cast h -> fp16 [100096, 128]; per core sort its 75k edges by
    (src>>15, dst>>15) into 16 bins so node indices fit int16 relative to a
    32768-row table base; pad bins to 128 (+last bin to make total %512);
    lay per-chunk indices as [16, L/16] int16 in an SBUF canvas.
  - Device (per core): for each 8192-edge slab:
      * transpose-mode dma_gather pulls h16 rows (256B) into
        guT/gvT [128 feat, slab] fp16 directly (xbar transpose in DMA;
        one instruction per multi-thousand-edge chunk -> SWDGE cheap)
      * per 512-edge group: psum[64, 512] = WuT.T@guT + WvT.T@gvT (+ b)
        with stationary [128, 64] weights, ACT copies psum -> fp16 score
      * one [64, slab] fp16 store (16 KB/partition descriptors)
  - Host: transpose [64, S_PAD] -> [S_PAD, 64], un-permute, cast f32.
"""

import numpy as np

import concourse.bacc as bacc
import concourse.bass as bass
import concourse.mybir as mybir
import concourse.tile as tile
from concourse.bass_utils import run_bass_kernel_spmd

N_CORES = 8
N_NODES = 100000
N_EDGES = 600000
D = 128
C = 64
N_NODES_PAD = 100096          # mult of 128
EDGES_PER_CORE = N_EDGES // N_CORES
BIN_SHIFT = 15                # 32768-node table windows (int16 idx)
N_SRC_BINS = (N_NODES >> BIN_SHIFT) + 1   # 4
N_BINS = N_SRC_BINS * N_SRC_BINS          # 16
SLAB = 7680                   # edges per score slab / gather target tile
MM = 512                      # edges per matmul group (one PSUM bank)
# One gather needs num_idxs/16 + 2 descriptor-ring slots per engine and the
# ring holds ~1024; 7680 -> 482 slots. Requires single_packet=False (a
# coalesced packet is limited to ~4KB/16 descs per engine).
MAX_GATHER = 7680
# Non-transpose (CME) gathers carry no xbar state, so they can spread across
# all 4 SWDGE queues for drain concurrency. (Transpose-mode gathers CANNOT:
# concurrent rx streams interleave per-descriptor and corrupt the xbar tile.)
N_QUEUES = 4

_F32 = mybir.dt.float32
_F16 = mybir.dt.float16
_I16 = mybir.dt.int16

_CACHE: dict = {}


def _plan_chunks(caps):
    """Static per-core slot plan from bin caps (same caps on all cores).

    Returns (s_pad, slabs, u_chunks, v_chunks, total_idx_cols) where each
    chunk is (slab_idx, dst_off_in_slab, length, table_base_row, idx_col0).
    Chunks never cross slab boundaries and are 128-aligned.
    """
    s_pad = int(sum(caps))
    assert s_pad % MM == 0
    bin_start = np.concatenate([[0], np.cumsum(caps)]).astype(int)

    slabs = []
    off = 0
    while off < s_pad:
        slabs.append(min(SLAB, s_pad - off))
        off += SLAB

    col = [0]

    def split(run_start, run_len, base_row):
        out = []
        pos = run_start
        end = run_start + run_len
        while pos < end:
            slab_i = pos // SLAB
            slab_end = (slab_i + 1) * SLAB
            ln = min(end, slab_end) - pos
            # cap at MAX_GATHER (SWDGE ring capacity), keep 128-aligned
            n_pieces = -(-ln // MAX_GATHER)
            piece = -(-(ln // 128) // n_pieces) * 128
            while ln > 0:
                p_ln = min(piece, ln)
                assert p_ln % 128 == 0 and pos % 128 == 0 and p_ln <= MAX_GATHER
                out.append((slab_i, pos - slab_i * SLAB, p_ln, base_row, col[0]))
                col[0] += p_ln // 16
                pos += p_ln
                ln -= p_ln
        return out

    u_chunks = []
    for bu in range(N_SRC_BINS):
        run_start = bin_start[bu * N_SRC_BINS]
        run_len = bin_start[(bu + 1) * N_SRC_BINS] - run_start
        if run_len:
            u_chunks += split(run_start, run_len, bu << BIN_SHIFT)
    v_chunks = []
    for k in range(N_BINS):
        if caps[k]:
            v_chunks += split(bin_start[k], caps[k], (k % N_SRC_BINS) << BIN_SHIFT)
    return s_pad, slabs, u_chunks, v_chunks, col[0]


def build_nc(caps, use_bias):
    key = (tuple(caps), use_bias)
    if key in _CACHE:
        return _CACHE[key]
    s_pad, slabs, u_chunks, v_chunks, idx_cols = _plan_chunks(caps)

    nc = bacc.Bacc("TRN2", target_bir_lowering=False, num_swdge_queues=N_QUEUES)
    h16 = nc.dram_tensor("h16", [N_NODES_PAD, D], _F16, kind="ExternalInput")
    idx = nc.dram_tensor("idx", [128, idx_cols], _I16, kind="ExternalInput")
    wut = nc.dram_tensor("wut", [D, C], _F16, kind="ExternalInput")
    wvt = nc.dram_tensor("wvt", [D, C], _F16, kind="ExternalInput")
    brow = nc.dram_tensor("brow", [1, C], _F16, kind="ExternalInput")
    out = nc.dram_tensor("out", [C, s_pad], _F16, kind="ExternalOutput")

    by_slab_u = {}
    for ch in u_chunks:
        by_slab_u.setdefault(ch[0], []).append(ch)
    by_slab_v = {}
    for ch in v_chunks:
        by_slab_v.setdefault(ch[0], []).append(ch)

    from concourse.masks import make_identity

    with tile.TileContext(nc) as tc:
        with (
            tc.tile_pool(name="const", bufs=1) as cpool,
            tc.tile_pool(name="gather", bufs=2) as gpool,
            tc.tile_pool(name="gt", bufs=4) as gtpool,
            tc.tile_pool(name="score", bufs=2) as spool,
            tc.tile_pool(name="psum_t", bufs=4, space="PSUM") as ptpool,
            tc.tile_pool(name="psum_s", bufs=2, space="PSUM") as pspool,
        ):
            ident = cpool.tile([128, 128], _F16)
            make_identity(nc, ident[:])
            wut_sb = cpool.tile([D, C], _F16)
            nc.sync.dma_start(out=wut_sb[:], in_=wut[:, :])
            wvt_sb = cpool.tile([D, C], _F16)
            nc.sync.dma_start(out=wvt_sb[:], in_=wvt[:, :])
            idx_sb = cpool.tile([128, idx_cols], _I16)
            nc.sync.dma_start(out=idx_sb[:], in_=idx[:, :])
            if use_bias:
                bias_sb = cpool.tile([1, C], _F16)
                nc.sync.dma_start(out=bias_sb[:], in_=brow[:, :])
                ones_sb = cpool.tile([1, MM], _F16)
                nc.vector.memset(ones_sb[:], 1.0)

            qn = 0
            cp = 0
            for s, slab_len in enumerate(slabs):
                gu = gpool.tile([128, SLAB // 128, D], _F16, tag="gu")
                gv = gpool.tile([128, SLAB // 128, D], _F16, tag="gv")
                for tgt, chunks in ((gu, by_slab_u[s]), (gv, by_slab_v[s])):
                    for (_si, doff, ln, base, c0) in chunks:
                        hi = min(base + (1 << BIN_SHIFT), N_NODES_PAD)
                        nc.gpsimd.dma_gather(
                            tgt[:, doff // 128:(doff + ln) // 128, :],
                            h16[base:hi, :],
                            idx_sb[:, c0:c0 + ln // 16],
                            ln,
                            ln,
                            D,
                            single_packet=False,
                            queue_num=qn,
                        )
                        qn = (qn + 1) % N_QUEUES
                score = spool.tile([C, SLAB], _F16, tag="sc")
                for b in range(slab_len // MM):
                    guT = gtpool.tile([128, MM], _F16, tag="gut")
                    gvT = gtpool.tile([128, MM], _F16, tag="gvt")
                    for j in range(MM // 128):
                        sub = b * (MM // 128) + j
                        for g_tile, gT in ((gu, guT), (gv, gvT)):
                            pt = ptpool.tile([128, 128], _F16, tag="pt")
                            nc.tensor.transpose(pt[:], g_tile[:, sub, :], ident[:])
                            dst = gT[:, j * 128:(j + 1) * 128]
                            if cp % 2 == 0:
                                nc.scalar.copy(out=dst, in_=pt[:])
                            else:
                                nc.vector.tensor_copy(dst, pt[:])
                            cp += 1
                    ps = pspool.tile([C, MM], _F32, tag="ps")
                    nc.tensor.matmul(
                        ps[:], lhsT=wut_sb[:], rhs=guT[:], start=True, stop=False,
                    )
                    nc.tensor.matmul(
                        ps[:], lhsT=wvt_sb[:], rhs=gvT[:],
                        start=False, stop=not use_bias,
                    )
                    if use_bias:
                        nc.tensor.matmul(
                            ps[:], lhsT=bias_sb[:], rhs=ones_sb[:],
                            start=False, stop=True,
                        )
                    nc.scalar.copy(out=score[:, b * MM:(b + 1) * MM], in_=ps[:])
                nc.sync.dma_start(
                    out=out[:, s * SLAB:s * SLAB + slab_len],
                    in_=score[:, :slab_len],
                )

    nc.finalize()
    _CACHE[key] = (nc, s_pad, u_chunks, v_chunks, idx_cols)
    return _CACHE[key]


def _bin_core(src_c, dst_c):
    bu = src_c >> BIN_SHIFT
    bv = dst_c >> BIN_SHIFT
    key = bu * N_SRC_BINS + bv
    order = np.argsort(key, kind="stable")
    counts = np.bincount(key, minlength=N_BINS)
    return order, counts


def make_in_maps(h, src, dst, W, b):
    h = np.asarray(h, dtype=np.float32)
    W = np.asarray(W, dtype=np.float32)
    b = np.asarray(b, dtype=np.float32)
    src = np.asarray(src).astype(np.int64)
    dst = np.asarray(dst).astype(np.int64)

    h16 = np.zeros((N_NODES_PAD, D), dtype=np.float16)
    h16[:N_NODES] = h.astype(np.float16)
    wut = np.ascontiguousarray(W[:, :D].T).astype(np.float16)
    wvt = np.ascontiguousarray(W[:, D:].T).astype(np.float16)
    brow = b.reshape(1, C).astype(np.float16)
    use_bias = bool(np.any(b))

    per_core = []
    max_counts = np.zeros(N_BINS, dtype=np.int64)
    for c in range(N_CORES):
        sl = slice(c * EDGES_PER_CORE, (c + 1) * EDGES_PER_CORE)
        order, counts = _bin_core(src[sl], dst[sl])
        per_core.append((order, counts))
        max_counts = np.maximum(max_counts, counts)

    caps = ((max_counts + 127) // 128) * 128
    total = int(caps.sum())
    pad_tail = (-total) % MM
    caps[-1] += pad_tail  # bin 15 always exists structurally
    caps = caps.astype(int)

    (nc, s_pad, u_chunks, v_chunks, idx_cols) = build_nc(tuple(caps), use_bias)
    bin_start = np.concatenate([[0], np.cumsum(caps)]).astype(int)

    in_maps = []
    orig_ids = []
    for c in range(N_CORES):
        e0 = c * EDGES_PER_CORE
        order, counts = per_core[c]
        src_c = src[e0:e0 + EDGES_PER_CORE][order]
        dst_c = dst[e0:e0 + EDGES_PER_CORE][order]
        key_sorted = (src_c >> BIN_SHIFT) * N_SRC_BINS + (dst_c >> BIN_SHIFT)

        u_slot = np.zeros(s_pad, dtype=np.int16)
        v_slot = np.zeros(s_pad, dtype=np.int16)
        oid = np.full(s_pad, -1, dtype=np.int64)
        pos = 0
        for k in range(N_BINS):
            n_k = int(counts[k])
            b0 = bin_start[k]
            u_slot[b0:b0 + n_k] = (src_c[pos:pos + n_k] & ((1 << BIN_SHIFT) - 1)).astype(np.int16)
            v_slot[b0:b0 + n_k] = (dst_c[pos:pos + n_k] & ((1 << BIN_SHIFT) - 1)).astype(np.int16)
            assert (key_sorted[pos:pos + n_k] == k).all()
            oid[b0:b0 + n_k] = e0 + order[pos:pos + n_k]
            pos += n_k

        canvas = np.zeros((16, idx_cols), dtype=np.int16)
        for chunks, slot in ((u_chunks, u_slot), (v_chunks, v_slot)):
            for (si, doff, ln, _base, c0) in chunks:
                g0 = si * SLAB + doff
                canvas[:, c0:c0 + ln // 16] = slot[g0:g0 + ln].reshape(ln // 16, 16).T
        # idx rows are read per-Q7-core from its own 16-partition group:
        # replicate the block across all 8 groups.
        canvas = np.tile(canvas, (8, 1))

        orig_ids.append(oid)
        in_maps.append(
            {"h16": h16, "idx": canvas, "wut": wut, "wvt": wvt, "brow": brow}
        )
    return nc, in_maps, orig_ids, s_pad


def assemble_output(results, orig_ids):
    final = np.empty((N_EDGES, C), dtype=np.float32)
    for res, oid in zip(results, orig_ids):
        scores = np.asarray(res["out"]).T.astype(np.float32)  # [s_pad, C]
        valid = oid >= 0
        final[oid[valid]] = scores[valid]
    return final


def run(h, src, dst, W, b, **spmd_kwargs):
    nc, in_maps, orig_ids, _ = make_in_maps(h, src, dst, W, b)
    res = run_bass_kernel_spmd(nc, in_maps, core_ids=list(range(N_CORES)), **spmd_kwargs)
    return assemble_output(res.results, orig_ids), res


def kernel(h, src, dst, W, b):
    out, _ = run(h, src, dst, W, b)
    return out


# revision 15
# speedup vs baseline: 3.4719x; 1.3783x over previous
"""GNN edge-scorer (MLPPredictor) Trainium2 kernel.

score[e, :] = h[src[e]] @ Wu.T + h[dst[e]] @ Wv.T + b
  h   [100000, 128] f32
  src/dst [600000] i64
  W   [64, 256] f32  (Wu = W[:, :128], Wv = W[:, 128:])
  out [600000, 64] f32

Strategy (8 cores, edge-sharded, fp16 internally):
  - Host: cast h -> fp16 [100096, 128]; per core sort its 75k edges by
    (src>>15, dst>>15) into 16 bins so node indices fit int16 relative to a
    32768-row table base; pad bins to 128 (+last bin to make total %512);
    lay per-chunk indices as [16, L/16] int16 in an SBUF canvas.
  - Device (per core): for each 8192-edge slab:
      * transpose-mode dma_gather pulls h16 rows (256B) into
        guT/gvT [128 feat, slab] fp16 directly (xbar transpose in DMA;
        one instruction per multi-thousand-edge chunk -> SWDGE cheap)
      * per 512-edge group: psum[64, 512] = WuT.T@guT + WvT.T@gvT (+ b)
        with stationary [128, 64] weights, ACT copies psum -> fp16 score
      * one [64, slab] fp16 store (16 KB/partition descriptors)
  - Host: transpose [64, S_PAD] -> [S_PAD, 64], un-permute, cast f32.
"""

import numpy as np

import concourse.bacc as bacc
import concourse.bass as bass
import concourse.mybir as mybir
import concourse.tile as tile
from concourse.bass_utils import run_bass_kernel_spmd

N_CORES = 8
N_NODES = 100000
N_EDGES = 600000
D = 128
C = 64
N_NODES_PAD = 100096          # mult of 128
EDGES_PER_CORE = N_EDGES // N_CORES
BIN_SHIFT = 15                # 32768-node table windows (int16 idx)
N_SRC_BINS = (N_NODES >> BIN_SHIFT) + 1   # 4
N_BINS = N_SRC_BINS * N_SRC_BINS          # 16
SLAB = 7680                   # edges per score slab / gather target tile
MM = 512                      # edges per matmul group (one PSUM bank)
# One gather needs num_idxs/16 + 2 descriptor-ring slots per engine and the
# ring holds ~1024; 7680 -> 482 slots. Requires single_packet=False (a
# coalesced packet is limited to ~4KB/16 descs per engine).
MAX_GATHER = 7680
# Non-transpose (CME) gathers carry no xbar state, so they can spread across
# all 4 SWDGE queues for drain concurrency. (Transpose-mode gathers CANNOT:
# concurrent rx streams interleave per-descriptor and corrupt the xbar tile.)
N_QUEUES = 4

_F32 = mybir.dt.float32
_F16 = mybir.dt.float16
_I16 = mybir.dt.int16

_CACHE: dict = {}


def _plan_chunks(caps):
    """Static per-core slot plan from bin caps (same caps on all cores).

    Returns (s_pad, slabs, u_chunks, v_chunks, total_idx_cols) where each
    chunk is (slab_idx, dst_off_in_slab, length, table_base_row, idx_col0).
    Chunks never cross slab boundaries and are 128-aligned.
    """
    s_pad = int(sum(caps))
    assert s_pad % MM == 0
    bin_start = np.concatenate([[0], np.cumsum(caps)]).astype(int)

    slabs = []
    off = 0
    while off < s_pad:
        slabs.append(min(SLAB, s_pad - off))
        off += SLAB

    col = [0]

    def split(run_start, run_len, base_row):
        out = []
        pos = run_start
        end = run_start + run_len
        while pos < end:
            slab_i = pos // SLAB
            slab_end = (slab_i + 1) * SLAB
            ln = min(end, slab_end) - pos
            # cap at MAX_GATHER (SWDGE ring capacity), keep 128-aligned
            n_pieces = -(-ln // MAX_GATHER)
            piece = -(-(ln // 128) // n_pieces) * 128
            while ln > 0:
                p_ln = min(piece, ln)
                assert p_ln % 128 == 0 and pos % 128 == 0 and p_ln <= MAX_GATHER
                out.append((slab_i, pos - slab_i * SLAB, p_ln, base_row, col[0]))
                col[0] += p_ln // 16
                pos += p_ln
                ln -= p_ln
        return out

    u_chunks = []
    for bu in range(N_SRC_BINS):
        run_start = bin_start[bu * N_SRC_BINS]
        run_len = bin_start[(bu + 1) * N_SRC_BINS] - run_start
        if run_len:
            u_chunks += split(run_start, run_len, bu << BIN_SHIFT)
    v_chunks = []
    for k in range(N_BINS):
        if caps[k]:
            v_chunks += split(bin_start[k], caps[k], (k % N_SRC_BINS) << BIN_SHIFT)
    return s_pad, slabs, u_chunks, v_chunks, col[0]


def build_nc(caps, use_bias):
    key = (tuple(caps), use_bias)
    if key in _CACHE:
        return _CACHE[key]
    s_pad, slabs, u_chunks, v_chunks, idx_cols = _plan_chunks(caps)

    nc = bacc.Bacc("TRN2", target_bir_lowering=False, num_swdge_queues=N_QUEUES)
    h16 = nc.dram_tensor("h16", [N_NODES_PAD, D], _F16, kind="ExternalInput")
    idx = nc.dram_tensor("idx", [128, idx_cols], _I16, kind="ExternalInput")
    wut = nc.dram_tensor("wut", [D, C], _F16, kind="ExternalInput")
    wvt = nc.dram_tensor("wvt", [D, C], _F16, kind="ExternalInput")
    brow = nc.dram_tensor("brow", [1, C], _F16, kind="ExternalInput")
    out = nc.dram_tensor("out", [C, s_pad], _F16, kind="ExternalOutput")

    by_slab_u = {}
    for ch in u_chunks:
        by_slab_u.setdefault(ch[0], []).append(ch)
    by_slab_v = {}
    for ch in v_chunks:
        by_slab_v.setdefault(ch[0], []).append(ch)

    from concourse.masks import make_identity

    with tile.TileContext(nc) as tc:
        with (
            tc.tile_pool(name="const", bufs=1) as cpool,
            tc.tile_pool(name="gather", bufs=3) as gpool,
            tc.tile_pool(name="gt", bufs=4) as gtpool,
            tc.tile_pool(name="score", bufs=2) as spool,
            tc.tile_pool(name="psum_t", bufs=4, space="PSUM") as ptpool,
            tc.tile_pool(name="psum_s", bufs=2, space="PSUM") as pspool,
        ):
            ident = cpool.tile([128, 128], _F16)
            make_identity(nc, ident[:])
            wut_sb = cpool.tile([D, C], _F16)
            nc.sync.dma_start(out=wut_sb[:], in_=wut[:, :])
            wvt_sb = cpool.tile([D, C], _F16)
            nc.sync.dma_start(out=wvt_sb[:], in_=wvt[:, :])
            idx_sb = cpool.tile([128, idx_cols], _I16)
            nc.sync.dma_start(out=idx_sb[:], in_=idx[:, :])
            if use_bias:
                bias_sb = cpool.tile([1, C], _F16)
                nc.sync.dma_start(out=bias_sb[:], in_=brow[:, :])
                ones_sb = cpool.tile([1, MM], _F16)
                nc.vector.memset(ones_sb[:], 1.0)

            qn = 0
            cp = 0
            for s, slab_len in enumerate(slabs):
                gu = gpool.tile([128, SLAB // 128, D], _F16, tag="gu")
                gv = gpool.tile([128, SLAB // 128, D], _F16, tag="gv")
                for tgt, chunks in ((gu, by_slab_u[s]), (gv, by_slab_v[s])):
                    for (_si, doff, ln, base, c0) in chunks:
                        hi = min(base + (1 << BIN_SHIFT), N_NODES_PAD)
                        nc.gpsimd.dma_gather(
                            tgt[:, doff // 128:(doff + ln) // 128, :],
                            h16[base:hi, :],
                            idx_sb[:, c0:c0 + ln // 16],
                            ln,
                            ln,
                            D,
                            single_packet=False,
                            queue_num=qn,
                        )
                        qn = (qn + 1) % N_QUEUES
                score = spool.tile([C, SLAB], _F16, tag="sc")
                for b in range(slab_len // MM):
                    guT = gtpool.tile([128, MM], _F16, tag="gut")
                    gvT = gtpool.tile([128, MM], _F16, tag="gvt")
                    for g_tile, gT in ((gu, guT), (gv, gvT)):
                        pt = ptpool.tile([128, MM], _F16, tag="pt")
                        for j in range(MM // 128):
                            sub = b * (MM // 128) + j
                            nc.tensor.transpose(
                                pt[:, j * 128:(j + 1) * 128], g_tile[:, sub, :], ident[:]
                            )
                        if cp % 2 == 0:
                            nc.scalar.copy(out=gT[:], in_=pt[:])
                        else:
                            nc.vector.tensor_copy(gT[:], pt[:])
                        cp += 1
                    ps = pspool.tile([C, MM], _F32, tag="ps")
                    nc.tensor.matmul(
                        ps[:], lhsT=wut_sb[:], rhs=guT[:], start=True, stop=False,
                    )
                    nc.tensor.matmul(
                        ps[:], lhsT=wvt_sb[:], rhs=gvT[:],
                        start=False, stop=not use_bias,
                    )
                    if use_bias:
                        nc.tensor.matmul(
                            ps[:], lhsT=bias_sb[:], rhs=ones_sb[:],
                            start=False, stop=True,
                        )
                    nc.scalar.copy(out=score[:, b * MM:(b + 1) * MM], in_=ps[:])
                nc.sync.dma_start(
                    out=out[:, s * SLAB:s * SLAB + slab_len],
                    in_=score[:, :slab_len],
                )

    nc.finalize()
    _CACHE[key] = (nc, s_pad, u_chunks, v_chunks, idx_cols)
    return _CACHE[key]


def _bin_core(src_c, dst_c):
    bu = src_c >> BIN_SHIFT
    bv = dst_c >> BIN_SHIFT
    key = bu * N_SRC_BINS + bv
    order = np.argsort(key, kind="stable")
    counts = np.bincount(key, minlength=N_BINS)
    return order, counts


def make_in_maps(h, src, dst, W, b):
    h = np.asarray(h, dtype=np.float32)
    W = np.asarray(W, dtype=np.float32)
    b = np.asarray(b, dtype=np.float32)
    src = np.asarray(src).astype(np.int64)
    dst = np.asarray(dst).astype(np.int64)

    h16 = np.zeros((N_NODES_PAD, D), dtype=np.float16)
    h16[:N_NODES] = h.astype(np.float16)
    wut = np.ascontiguousarray(W[:, :D].T).astype(np.float16)
    wvt = np.ascontiguousarray(W[:, D:].T).astype(np.float16)
    brow = b.reshape(1, C).astype(np.float16)
    use_bias = bool(np.any(b))

    per_core = []
    max_counts = np.zeros(N_BINS, dtype=np.int64)
    for c in range(N_CORES):
        sl = slice(c * EDGES_PER_CORE, (c + 1) * EDGES_PER_CORE)
        order, counts = _bin_core(src[sl], dst[sl])
        per_core.append((order, counts))
        max_counts = np.maximum(max_counts, counts)

    caps = ((max_counts + 127) // 128) * 128
    total = int(caps.sum())
    pad_tail = (-total) % MM
    caps[-1] += pad_tail  # bin 15 always exists structurally
    caps = caps.astype(int)

    (nc, s_pad, u_chunks, v_chunks, idx_cols) = build_nc(tuple(caps), use_bias)
    bin_start = np.concatenate([[0], np.cumsum(caps)]).astype(int)

    in_maps = []
    orig_ids = []
    for c in range(N_CORES):
        e0 = c * EDGES_PER_CORE
        order, counts = per_core[c]
        src_c = src[e0:e0 + EDGES_PER_CORE][order]
        dst_c = dst[e0:e0 + EDGES_PER_CORE][order]
        key_sorted = (src_c >> BIN_SHIFT) * N_SRC_BINS + (dst_c >> BIN_SHIFT)

        u_slot = np.zeros(s_pad, dtype=np.int16)
        v_slot = np.zeros(s_pad, dtype=np.int16)
        oid = np.full(s_pad, -1, dtype=np.int64)
        pos = 0
        for k in range(N_BINS):
            n_k = int(counts[k])
            b0 = bin_start[k]
            u_slot[b0:b0 + n_k] = (src_c[pos:pos + n_k] & ((1 << BIN_SHIFT) - 1)).astype(np.int16)
            v_slot[b0:b0 + n_k] = (dst_c[pos:pos + n_k] & ((1 << BIN_SHIFT) - 1)).astype(np.int16)
            assert (key_sorted[pos:pos + n_k] == k).all()
            oid[b0:b0 + n_k] = e0 + order[pos:pos + n_k]
            pos += n_k

        canvas = np.zeros((16, idx_cols), dtype=np.int16)
        for chunks, slot in ((u_chunks, u_slot), (v_chunks, v_slot)):
            for (si, doff, ln, _base, c0) in chunks:
                g0 = si * SLAB + doff
                canvas[:, c0:c0 + ln // 16] = slot[g0:g0 + ln].reshape(ln // 16, 16).T
        # idx rows are read per-Q7-core from its own 16-partition group:
        # replicate the block across all 8 groups.
        canvas = np.tile(canvas, (8, 1))

        orig_ids.append(oid)
        in_maps.append(
            {"h16": h16, "idx": canvas, "wut": wut, "wvt": wvt, "brow": brow}
        )
    return nc, in_maps, orig_ids, s_pad


def assemble_output(results, orig_ids):
    final = np.empty((N_EDGES, C), dtype=np.float32)
    for res, oid in zip(results, orig_ids):
        scores = np.asarray(res["out"]).T.astype(np.float32)  # [s_pad, C]
        valid = oid >= 0
        final[oid[valid]] = scores[valid]
    return final


def run(h, src, dst, W, b, **spmd_kwargs):
    nc, in_maps, orig_ids, _ = make_in_maps(h, src, dst, W, b)
    res = run_bass_kernel_spmd(nc, in_maps, core_ids=list(range(N_CORES)), **spmd_kwargs)
    return assemble_output(res.results, orig_ids), res


def kernel(h, src, dst, W, b):
    out, _ = run(h, src, dst, W, b)
    return out


# revision 16
# speedup vs baseline: 4.1328x; 1.1904x over previous
"""GNN edge-scorer (MLPPredictor) Trainium2 kernel.

score[e, :] = h[src[e]] @ Wu.T + h[dst[e]] @ Wv.T + b
  h   [100000, 128] f32
  src/dst [600000] i64
  W   [64, 256] f32  (Wu = W[:, :128], Wv = W[:, 128:])
  out [600000, 64] f32

Strategy (8 cores, edge-sharded, fp16 internally):
  - Host: cast h -> fp16 [100096, 128]; per core sort its 75k edges by
    (src>>15, dst>>15) into 16 bins so node indices fit int16 relative to a
    32768-row table base; pad bins to 128 (+last bin to make total %512);
    lay per-chunk indices as [16, L/16] int16 in an SBUF canvas.
  - Device (per core): for each 8192-edge slab:
      * transpose-mode dma_gather pulls h16 rows (256B) into
        guT/gvT [128 feat, slab] fp16 directly (xbar transpose in DMA;
        one instruction per multi-thousand-edge chunk -> SWDGE cheap)
      * per 512-edge group: psum[64, 512] = WuT.T@guT + WvT.T@gvT (+ b)
        with stationary [128, 64] weights, ACT copies psum -> fp16 score
      * one [64, slab] fp16 store (16 KB/partition descriptors)
  - Host: transpose [64, S_PAD] -> [S_PAD, 64], un-permute, cast f32.
"""

import numpy as np

import concourse.bacc as bacc
import concourse.bass as bass
import concourse.mybir as mybir
import concourse.tile as tile
from concourse.bass_utils import run_bass_kernel_spmd

N_CORES = 8
N_NODES = 100000
N_EDGES = 600000
D = 128
C = 64
N_NODES_PAD = 100096          # mult of 128
EDGES_PER_CORE = N_EDGES // N_CORES
BIN_SHIFT = 15                # 32768-node table windows (int16 idx)
N_SRC_BINS = (N_NODES >> BIN_SHIFT) + 1   # 4
N_BINS = N_SRC_BINS * N_SRC_BINS          # 16
SLAB = 7680                   # edges per score slab / gather target tile
MM = 512                      # edges per matmul group (one PSUM bank)
# One gather needs num_idxs/16 + 2 descriptor-ring slots per engine and the
# ring holds ~1024; 7680 -> 482 slots. Requires single_packet=False (a
# coalesced packet is limited to ~4KB/16 descs per engine).
MAX_GATHER = 3840
# Non-transpose (CME) gathers carry no xbar state, so they can spread across
# all 4 SWDGE queues for drain concurrency. (Transpose-mode gathers CANNOT:
# concurrent rx streams interleave per-descriptor and corrupt the xbar tile.)
N_QUEUES = 4

_F32 = mybir.dt.float32
_F16 = mybir.dt.float16
_I16 = mybir.dt.int16

_CACHE: dict = {}


def _plan_chunks(caps):
    """Static per-core slot plan from bin caps (same caps on all cores).

    Returns (s_pad, slabs, u_chunks, v_chunks, total_idx_cols) where each
    chunk is (slab_idx, dst_off_in_slab, length, table_base_row, idx_col0).
    Chunks never cross slab boundaries and are 128-aligned.
    """
    s_pad = int(sum(caps))
    assert s_pad % MM == 0
    bin_start = np.concatenate([[0], np.cumsum(caps)]).astype(int)

    slabs = []
    off = 0
    while off < s_pad:
        slabs.append(min(SLAB, s_pad - off))
        off += SLAB

    col = [0]

    def split(run_start, run_len, base_row):
        out = []
        pos = run_start
        end = run_start + run_len
        while pos < end:
            slab_i = pos // SLAB
            slab_end = (slab_i + 1) * SLAB
            ln = min(end, slab_end) - pos
            # cap at MAX_GATHER (SWDGE ring capacity), keep 128-aligned
            n_pieces = -(-ln // MAX_GATHER)
            piece = -(-(ln // 128) // n_pieces) * 128
            while ln > 0:
                p_ln = min(piece, ln)
                assert p_ln % 128 == 0 and pos % 128 == 0 and p_ln <= MAX_GATHER
                out.append((slab_i, pos - slab_i * SLAB, p_ln, base_row, col[0]))
                col[0] += p_ln // 16
                pos += p_ln
                ln -= p_ln
        return out

    u_chunks = []
    for bu in range(N_SRC_BINS):
        run_start = bin_start[bu * N_SRC_BINS]
        run_len = bin_start[(bu + 1) * N_SRC_BINS] - run_start
        if run_len:
            u_chunks += split(run_start, run_len, bu << BIN_SHIFT)
    v_chunks = []
    for k in range(N_BINS):
        if caps[k]:
            v_chunks += split(bin_start[k], caps[k], (k % N_SRC_BINS) << BIN_SHIFT)
    return s_pad, slabs, u_chunks, v_chunks, col[0]


def build_nc(caps, use_bias):
    key = (tuple(caps), use_bias)
    if key in _CACHE:
        return _CACHE[key]
    s_pad, slabs, u_chunks, v_chunks, idx_cols = _plan_chunks(caps)

    nc = bacc.Bacc("TRN2", target_bir_lowering=False, num_swdge_queues=N_QUEUES)
    h16 = nc.dram_tensor("h16", [N_NODES_PAD, D], _F16, kind="ExternalInput")
    idx = nc.dram_tensor("idx", [128, idx_cols], _I16, kind="ExternalInput")
    wut = nc.dram_tensor("wut", [D, C], _F16, kind="ExternalInput")
    wvt = nc.dram_tensor("wvt", [D, C], _F16, kind="ExternalInput")
    brow = nc.dram_tensor("brow", [1, C], _F16, kind="ExternalInput")
    out = nc.dram_tensor("out", [C, s_pad], _F16, kind="ExternalOutput")

    by_slab_u = {}
    for ch in u_chunks:
        by_slab_u.setdefault(ch[0], []).append(ch)
    by_slab_v = {}
    for ch in v_chunks:
        by_slab_v.setdefault(ch[0], []).append(ch)

    from concourse.masks import make_identity

    with tile.TileContext(nc) as tc:
        with (
            tc.tile_pool(name="const", bufs=1) as cpool,
            tc.tile_pool(name="gather", bufs=3) as gpool,
            tc.tile_pool(name="gt", bufs=4) as gtpool,
            tc.tile_pool(name="score", bufs=2) as spool,
            tc.tile_pool(name="psum_t", bufs=4, space="PSUM") as ptpool,
            tc.tile_pool(name="psum_s", bufs=2, space="PSUM") as pspool,
        ):
            ident = cpool.tile([128, 128], _F16)
            make_identity(nc, ident[:])
            wut_sb = cpool.tile([D, C], _F16)
            nc.sync.dma_start(out=wut_sb[:], in_=wut[:, :])
            wvt_sb = cpool.tile([D, C], _F16)
            nc.sync.dma_start(out=wvt_sb[:], in_=wvt[:, :])
            idx_sb = cpool.tile([128, idx_cols], _I16)
            nc.sync.dma_start(out=idx_sb[:], in_=idx[:, :])
            if use_bias:
                bias_sb = cpool.tile([1, C], _F16)
                nc.sync.dma_start(out=bias_sb[:], in_=brow[:, :])
                ones_sb = cpool.tile([1, MM], _F16)
                nc.vector.memset(ones_sb[:], 1.0)

            qn = 0
            cp = 0
            for s, slab_len in enumerate(slabs):
                gu = gpool.tile([128, SLAB // 128, D], _F16, tag="gu")
                gv = gpool.tile([128, SLAB // 128, D], _F16, tag="gv")
                for tgt, chunks in ((gu, by_slab_u[s]), (gv, by_slab_v[s])):
                    for (_si, doff, ln, base, c0) in chunks:
                        hi = min(base + (1 << BIN_SHIFT), N_NODES_PAD)
                        nc.gpsimd.dma_gather(
                            tgt[:, doff // 128:(doff + ln) // 128, :],
                            h16[base:hi, :],
                            idx_sb[:, c0:c0 + ln // 16],
                            ln,
                            ln,
                            D,
                            single_packet=False,
                            queue_num=qn,
                        )
                        qn = (qn + 1) % N_QUEUES
                score = spool.tile([C, SLAB], _F16, tag="sc")
                for b in range(slab_len // MM):
                    guT = gtpool.tile([128, MM], _F16, tag="gut")
                    gvT = gtpool.tile([128, MM], _F16, tag="gvt")
                    for g_tile, gT in ((gu, guT), (gv, gvT)):
                        pt = ptpool.tile([128, MM], _F16, tag="pt")
                        for j in range(MM // 128):
                            sub = b * (MM // 128) + j
                            nc.tensor.transpose(
                                pt[:, j * 128:(j + 1) * 128], g_tile[:, sub, :], ident[:]
                            )
                        if cp % 2 == 0:
                            nc.scalar.copy(out=gT[:], in_=pt[:])
                        else:
                            nc.vector.tensor_copy(gT[:], pt[:])
                        cp += 1
                    ps = pspool.tile([C, MM], _F32, tag="ps")
                    nc.tensor.matmul(
                        ps[:], lhsT=wut_sb[:], rhs=guT[:], start=True, stop=False,
                    )
                    nc.tensor.matmul(
                        ps[:], lhsT=wvt_sb[:], rhs=gvT[:],
                        start=False, stop=not use_bias,
                    )
                    if use_bias:
                        nc.tensor.matmul(
                            ps[:], lhsT=bias_sb[:], rhs=ones_sb[:],
                            start=False, stop=True,
                        )
                    nc.scalar.copy(out=score[:, b * MM:(b + 1) * MM], in_=ps[:])
                nc.sync.dma_start(
                    out=out[:, s * SLAB:s * SLAB + slab_len],
                    in_=score[:, :slab_len],
                )

    nc.finalize()
    _CACHE[key] = (nc, s_pad, u_chunks, v_chunks, idx_cols)
    return _CACHE[key]


def _bin_core(src_c, dst_c):
    bu = src_c >> BIN_SHIFT
    bv = dst_c >> BIN_SHIFT
    key = bu * N_SRC_BINS + bv
    order = np.argsort(key, kind="stable")
    counts = np.bincount(key, minlength=N_BINS)
    return order, counts


def make_in_maps(h, src, dst, W, b):
    h = np.asarray(h, dtype=np.float32)
    W = np.asarray(W, dtype=np.float32)
    b = np.asarray(b, dtype=np.float32)
    src = np.asarray(src).astype(np.int64)
    dst = np.asarray(dst).astype(np.int64)

    h16 = np.zeros((N_NODES_PAD, D), dtype=np.float16)
    h16[:N_NODES] = h.astype(np.float16)
    wut = np.ascontiguousarray(W[:, :D].T).astype(np.float16)
    wvt = np.ascontiguousarray(W[:, D:].T).astype(np.float16)
    brow = b.reshape(1, C).astype(np.float16)
    use_bias = bool(np.any(b))

    per_core = []
    max_counts = np.zeros(N_BINS, dtype=np.int64)
    for c in range(N_CORES):
        sl = slice(c * EDGES_PER_CORE, (c + 1) * EDGES_PER_CORE)
        order, counts = _bin_core(src[sl], dst[sl])
        per_core.append((order, counts))
        max_counts = np.maximum(max_counts, counts)

    caps = ((max_counts + 127) // 128) * 128
    total = int(caps.sum())
    pad_tail = (-total) % MM
    caps[-1] += pad_tail  # bin 15 always exists structurally
    caps = caps.astype(int)

    (nc, s_pad, u_chunks, v_chunks, idx_cols) = build_nc(tuple(caps), use_bias)
    bin_start = np.concatenate([[0], np.cumsum(caps)]).astype(int)

    in_maps = []
    orig_ids = []
    for c in range(N_CORES):
        e0 = c * EDGES_PER_CORE
        order, counts = per_core[c]
        src_c = src[e0:e0 + EDGES_PER_CORE][order]
        dst_c = dst[e0:e0 + EDGES_PER_CORE][order]
        key_sorted = (src_c >> BIN_SHIFT) * N_SRC_BINS + (dst_c >> BIN_SHIFT)

        u_slot = np.zeros(s_pad, dtype=np.int16)
        v_slot = np.zeros(s_pad, dtype=np.int16)
        oid = np.full(s_pad, -1, dtype=np.int64)
        pos = 0
        for k in range(N_BINS):
            n_k = int(counts[k])
            b0 = bin_start[k]
            u_slot[b0:b0 + n_k] = (src_c[pos:pos + n_k] & ((1 << BIN_SHIFT) - 1)).astype(np.int16)
            v_slot[b0:b0 + n_k] = (dst_c[pos:pos + n_k] & ((1 << BIN_SHIFT) - 1)).astype(np.int16)
            assert (key_sorted[pos:pos + n_k] == k).all()
            oid[b0:b0 + n_k] = e0 + order[pos:pos + n_k]
            pos += n_k

        canvas = np.zeros((16, idx_cols), dtype=np.int16)
        for chunks, slot in ((u_chunks, u_slot), (v_chunks, v_slot)):
            for (si, doff, ln, _base, c0) in chunks:
                g0 = si * SLAB + doff
                canvas[:, c0:c0 + ln // 16] = slot[g0:g0 + ln].reshape(ln // 16, 16).T
        # idx rows are read per-Q7-core from its own 16-partition group:
        # replicate the block across all 8 groups.
        canvas = np.tile(canvas, (8, 1))

        orig_ids.append(oid)
        in_maps.append(
            {"h16": h16, "idx": canvas, "wut": wut, "wvt": wvt, "brow": brow}
        )
    return nc, in_maps, orig_ids, s_pad


def assemble_output(results, orig_ids):
    final = np.empty((N_EDGES, C), dtype=np.float32)
    for res, oid in zip(results, orig_ids):
        scores = np.asarray(res["out"]).T.astype(np.float32)  # [s_pad, C]
        valid = oid >= 0
        final[oid[valid]] = scores[valid]
    return final


def run(h, src, dst, W, b, **spmd_kwargs):
    nc, in_maps, orig_ids, _ = make_in_maps(h, src, dst, W, b)
    res = run_bass_kernel_spmd(nc, in_maps, core_ids=list(range(N_CORES)), **spmd_kwargs)
    return assemble_output(res.results, orig_ids), res


def kernel(h, src, dst, W, b):
    out, _ = run(h, src, dst, W, b)
    return out
